# revision 2
# baseline (speedup 1.0000x reference)
"""Multi-head attention (N=4, T=2048, D=512, H=8, dh=64) on 8 TRN2 NeuronCores.

Sharding: batch N (4) x head-group (2 groups of 4 heads) -> 8 cores.

v2 structure (per core, heads 0..3 of its group; super-pair sp = (qb, t2)
covers q-block qb (512 queries) and the two heads in kT/qT tile t2):
  - scores: K=64 matmuls row-packed: head-lo on PE tile T0 (partitions
    0-63), head-hi on T8 (64-127), interleaved so they run concurrently.
  - exp: split between ScalarE (native Exp, 1024-wide PSUM chunks) and
    VectorE (Schraudolph bit-trick: bf16 = bitcast_i16(round(A*s + B)),
    one fused tensor_scalar op) on disjoint k-tile ranges.
  - AV: [V | 1]^T @ P accumulated per head into PSUM [65, 512]; row 64 is
    the softmax denominator. Unnormalized [65, 512] blocks are copied to
    SBUF and DMA'd out; the HOST does the division + transpose.
"""

import math

import ml_dtypes
import numpy as np

import concourse.bass as bass
import concourse.mybir as mybir
import concourse.tile as tile
from concourse import bacc
from concourse.bass_utils import run_bass_kernel_spmd

F32 = mybir.dt.float32
BF16 = mybir.dt.bfloat16
I16 = mybir.dt.int16
EXP = mybir.ActivationFunctionType.Exp
MULT = mybir.AluOpType.mult
ADD = mybir.AluOpType.add

N, T, D = 4, 2048, 512
HPC, DH = 4, 64          # heads per core, head dim
GC = HPC * DH            # head-group columns (256)
SCALE = 1.0 / math.sqrt(D)
QB = 512                 # q block
NQB = T // QB            # 4
NKT = T // 128           # 16 k tiles
KS = D // 128            # 4 contraction slices for projections
OROW = DH + 1            # 65 output rows per head (64 dh + denominator)

# exp engine split per head: k-tiles [0, DVE_KT) on VectorE (Schraudolph),
# k-tiles [DVE_KT, 16) on ScalarE in 2-kt (1024-wide) chunks.
DVE_KT = 6
# Schraudolph constants: bf16 = bitcast_i16(round_i16(raw_score*SCH_A + SCH_B))
SCH_C = 4.0
SCH_A = (128.0 / math.log(2.0)) * SCALE
SCH_B = 127.0 * 128.0 - SCH_C


def build():
    nc = bacc.Bacc("TRN2", target_bir_lowering=False, debug=False, num_devices=8)
    qT_in = nc.declare_dram_parameter("qT", [D, T], BF16, isOutput=False)
    kT_in = nc.declare_dram_parameter("kT", [D, T], BF16, isOutput=False)
    wq_in = nc.declare_dram_parameter("wq", [D, GC], BF16, isOutput=False)
    wk_in = nc.declare_dram_parameter("wk", [D, GC], BF16, isOutput=False)
    wv_in = nc.declare_dram_parameter("wv", [D, GC], BF16, isOutput=False)
    oT_out = nc.declare_dram_parameter("oT65", [HPC * OROW, T], F32, isOutput=True)

    with tile.TileContext(nc) as tc:
        with (
            tc.tile_pool(name="stage", bufs=8) as stage,
            tc.tile_pool(name="const", bufs=1) as const,
            tc.tile_pool(name="act", bufs=1) as actp,
            tc.tile_pool(name="ptl", bufs=2) as ptlp,
            tc.tile_pool(name="pth", bufs=2) as pthp,
            tc.tile_pool(name="ost", bufs=4) as ostp,
            tc.tile_pool(name="psS", bufs=2, space="PSUM") as psS,   # 4 banks
            tc.tile_pool(name="psD", bufs=2, space="PSUM") as psD,   # 2 banks
            tc.tile_pool(name="psC", bufs=2, space="PSUM") as psC,   # 2 banks
        ):
            # ---- weights (wk first: k-projection is the critical path) ----
            ws = {}
            for nm, src in (("wk", wk_in), ("wq", wq_in), ("wv", wv_in)):
                w = const.tile([128, KS, GC], BF16, tag=nm)
                nc.sync.dma_start(w[:], src.rearrange("(s p) c -> p s c", p=128))
                ws[nm] = w

            # ---- key^T staging (all of it, before q: gates first scores) ----
            kin = []
            for s in range(KS):
                t_ = stage.tile([128, T], BF16, tag="qkin", name=f"kin{s}")
                kin.append(t_)
            for tb in range(NQB):
                for s in range(KS):
                    nc.sync.dma_start(
                        kin[s][:, tb * QB : (tb + 1) * QB],
                        kT_in[s * 128 : (s + 1) * 128, tb * QB : (tb + 1) * QB],
                    )

            # ---- query^T staging (qb0 first) ----
            qin = []
            for s in range(KS):
                t_ = stage.tile([128, T], BF16, tag="qkin", name=f"qin{s}")
                qin.append(t_)
            for tb in range(NQB):
                for s in range(KS):
                    nc.sync.dma_start(
                        qin[s][:, tb * QB : (tb + 1) * QB],
                        qT_in[s * 128 : (s + 1) * 128, tb * QB : (tb + 1) * QB],
                    )

            kT_att = [
                actp.tile([128, T], BF16, tag=f"ka{d}", name=f"ka{d}")
                for d in range(2)
            ]
            qT_att = [
                actp.tile([128, T], BF16, tag=f"qa{d}", name=f"qa{d}")
                for d in range(2)
            ]

            # ---- kT projection: t2=0 first (first super-pair needs it) ----
            def emit_kproj(dt2):
                for tb in range(NQB):
                    ps = psD.tile([128, QB], F32, tag="D", name=f"kproj{dt2}_{tb}")
                    for s in range(KS):
                        nc.tensor.matmul(
                            ps[:],
                            ws["wk"][:, s, dt2 * 128 : (dt2 + 1) * 128],
                            kin[s][:, tb * QB : (tb + 1) * QB],
                            start=(s == 0),
                            stop=(s == KS - 1),
                        )
                    nc.vector.tensor_copy(
                        kT_att[dt2][:, tb * QB : (tb + 1) * QB], ps[:]
                    )

            def emit_qproj(qb):
                for dt2 in range(2):
                    ps = psD.tile([128, QB], F32, tag="D", name=f"qproj{qb}_{dt2}")
                    for s in range(KS):
                        nc.tensor.matmul(
                            ps[:],
                            ws["wq"][:, s, dt2 * 128 : (dt2 + 1) * 128],
                            qin[s][:, qb * QB : (qb + 1) * QB],
                            start=(s == 0),
                            stop=(s == KS - 1),
                        )
                    nc.vector.tensor_copy(
                        qT_att[dt2][:, qb * QB : (qb + 1) * QB], ps[:]
                    )

            # ---- V projection into [128, kt, head, 65] with ones column ----
            vp = const.tile([128, NKT, HPC, OROW], BF16, tag="vp")
            ones_f32 = const.tile([128, NKT * HPC], F32, tag="ones")
            nc.gpsimd.memset(ones_f32[:], 1.0)
            nc.vector.tensor_copy(
                vp[:, :, :, DH : DH + 1],
                ones_f32[:].rearrange("p (a b) -> p a b", b=HPC).unsqueeze(3),
            )

            def emit_vproj(tt):
                ps = psD.tile([128, QB], F32, tag="D", name=f"vproj{tt}")
                for s in range(KS):
                    nc.tensor.matmul(
                        ps[:, 0:GC],
                        kin[s][:, tt * 128 : (tt + 1) * 128],
                        ws["wv"][:, s, :],
                        start=(s == 0),
                        stop=(s == KS - 1),
                    )
                nc.vector.tensor_copy(
                    vp[:, tt, :, 0:DH],
                    ps[:, 0:GC].rearrange("p (h d) -> p h d", d=DH),
                )

            emit_kproj(0)
            emit_qproj(0)

            # ---- attention super-pairs ----
            def emit_scores(t2, qb, pt_lo, pt_hi, sp_idx):
                """Row-packed score matmuls + split exp for both heads."""
                q_lo = qT_att[t2][0:DH, qb * QB : (qb + 1) * QB]
                q_hi = qT_att[t2][DH:128, qb * QB : (qb + 1) * QB]

                # DVE chunks first (1 kt each), interleaved lo/hi
                for kt in range(DVE_KT):
                    pl = psD.tile([128, QB], F32, tag="D", name=f"sD_lo{kt}")
                    ph = psD.tile([128, QB], F32, tag="D", name=f"sD_hi{kt}")
                    nc.tensor.matmul(
                        pl[:],
                        kT_att[t2][0:DH, kt * 128 : (kt + 1) * 128],
                        q_lo, start=True, stop=True,
                    )
                    nc.tensor.matmul(
                        ph[:],
                        kT_att[t2][DH:128, kt * 128 : (kt + 1) * 128],
                        q_hi, start=True, stop=True,
                    )
                    nc.vector.tensor_scalar(
                        pt_lo[:, kt * QB : (kt + 1) * QB].bitcast(I16),
                        pl[:], SCH_A, SCH_B, MULT, ADD,
                    )
                    nc.vector.tensor_scalar(
                        pt_hi[:, kt * QB : (kt + 1) * QB].bitcast(I16),
                        ph[:], SCH_A, SCH_B, MULT, ADD,
                    )

                # ScalarE chunks: 2 kt (1024 wide) per activation call
                for c in range((NKT - DVE_KT) // 2):
                    kt0 = DVE_KT + 2 * c
                    for which, qsrc, pt in (("lo", q_lo, pt_lo), ("hi", q_hi, pt_hi)):
                        base = 0 if which == "lo" else DH
                        ps = psS.tile([128, 2 * QB], F32, tag="S",
                                      name=f"sS_{which}{c}")
                        for l in range(2):
                            kt = kt0 + l
                            nc.tensor.matmul(
                                ps[:, l * QB : (l + 1) * QB],
                                kT_att[t2][base : base + DH,
                                           kt * 128 : (kt + 1) * 128],
                                qsrc, start=True, stop=True,
                            )
                        nc.scalar.activation(
                            pt[:, kt0 * QB : (kt0 + 2) * QB],
                            ps[:], EXP, scale=SCALE,
                        )

            def emit_av(t2, qb, pt_lo, pt_hi, po_lo, po_hi):
                hp_lo, hp_hi = 2 * t2, 2 * t2 + 1
                for kt in range(NKT):
                    nc.tensor.matmul(
                        po_lo[0:OROW],
                        vp[:, kt, hp_lo, :],
                        pt_lo[:, kt * QB : (kt + 1) * QB],
                        start=(kt == 0), stop=(kt == NKT - 1),
                    )
                    nc.tensor.matmul(
                        po_hi[0:OROW],
                        vp[:, kt, hp_hi, :],
                        pt_hi[:, kt * QB : (kt + 1) * QB],
                        start=(kt == 0), stop=(kt == NKT - 1),
                    )

            def emit_out(t2, qb, po, which):
                hp = 2 * t2 + (0 if which == "lo" else 1)
                st = ostp.tile([128, QB], F32, tag="ost", name=f"o{which}")
                if which == "lo":
                    nc.scalar.copy(st[0:OROW, :], po[0:OROW, :])
                else:
                    nc.vector.tensor_copy(st[0:OROW, :], po[0:OROW, :])
                nc.gpsimd.dma_start(
                    oT_out[hp * OROW : (hp + 1) * OROW,
                           qb * QB : (qb + 1) * QB],
                    st[0:OROW, :],
                )

            sps = [(qb, t2) for qb in range(NQB) for t2 in range(2)]
            for i, (qb, t2) in enumerate(sps):
                pt_lo = ptlp.tile([128, NKT * QB], BF16, tag="ptl", name="ptl")
                pt_hi = pthp.tile([128, NKT * QB], BF16, tag="pth", name="pth")
                emit_scores(t2, qb, pt_lo, pt_hi, i)
                # prologue work folded into the first super-pair's slack
                if i == 0:
                    emit_kproj(1)
                    for tt in range(NKT):
                        emit_vproj(tt)
                if t2 == 1 and qb + 1 < NQB:
                    emit_qproj(qb + 1)
                po_lo = psC.tile([128, QB], F32, tag="C", name="po_lo")
                po_hi = psC.tile([128, QB], F32, tag="C", name="po_hi")
                emit_av(t2, qb, pt_lo, pt_hi, po_lo, po_hi)
                emit_out(t2, qb, po_lo, "lo")
                emit_out(t2, qb, po_hi, "hi")

    nc.compile()
    return nc


_NC = None


def _get_nc():
    global _NC
    if _NC is None:
        _NC = build()
    return _NC


def run(query, key, W_query, W_key, W_value, trace=False):
    nc = _get_nc()
    query = np.asarray(query, dtype=np.float32)
    key = np.asarray(key, dtype=np.float32)
    W_query = np.asarray(W_query, dtype=np.float32)
    W_key = np.asarray(W_key, dtype=np.float32)
    W_value = np.asarray(W_value, dtype=np.float32)

    in_maps = []
    for c in range(8):
        n, g = c // 2, c % 2
        cols = slice(g * GC, (g + 1) * GC)
        in_maps.append(
            {
                "qT": np.ascontiguousarray(query[n].T.astype(ml_dtypes.bfloat16)),
                "kT": np.ascontiguousarray(key[n].T.astype(ml_dtypes.bfloat16)),
                "wq": np.ascontiguousarray(W_query[:, cols].astype(ml_dtypes.bfloat16)),
                "wk": np.ascontiguousarray(W_key[:, cols].astype(ml_dtypes.bfloat16)),
                "wv": np.ascontiguousarray(W_value[:, cols].astype(ml_dtypes.bfloat16)),
            }
        )
    res = run_bass_kernel_spmd(nc, in_maps, core_ids=list(range(8)), trace=trace)
    out = np.empty((N, T, D), dtype=np.float32)
    for c in range(8):
        n, g = c // 2, c % 2
        r = res.results[c]["oT65"]  # [4*65, 2048]
        for hp in range(HPC):
            blk = r[hp * OROW : (hp + 1) * OROW]
            out[n, :, g * GC + hp * DH : g * GC + (hp + 1) * DH] = (
                blk[0:DH] / blk[DH : DH + 1]
            ).T
    return out, res


def kernel(query, key, W_query, W_key, W_value):
    out, _ = run(query, key, W_query, W_key, W_value, trace=False)
    return out
